# revision 47
# baseline (speedup 1.0000x reference)
"""MultiHeadAttn (post-LN, key-padding mask) Trainium2 Bass kernel, 8 cores.

Problem: h [S=2048, B=4, D=1024] f32; 16 heads x 64; key-padding mask [S, B];
out = LayerNorm(h + MHA(h)).

Sharding: core c handles batch b = c//2 and query half qh = c%2 (1024 query
rows), with all 16 heads and the full 2048-key context for that batch.
KV projections are recomputed by both cores of a batch pair (no collectives).

Per-core device pipeline (all matmuls bf16, fp32 accumulation in PSUM):
  - K^T/Q^T proj: stationary Wk/Wq column tiles, moving h^T -> [e, t] layout.
  - V proj: stationary h^T tiles, moving Wv -> natural [t, e] layout, stored
    as [V_h | ones64] per head (128-wide stationary).
  - Attention per head pair: scores^T [j,i] via row-paired matmuls (two heads
    in row strips 0-63 / 64-127 of the PE array), exp via ScalarE with the
    key-padding bias as a per-partition bias and 1/sqrt(dh) as the scale,
    PV with the [V|ones] stationary (M=128: rows 0-63 attn_vec, rows 64-127
    the softmax denominator replicated), then normalize via cross-partition
    copy + reciprocal_approx_fast + one stt per head.
  - Output proj: stationary attn_vec^T tiles, moving Wo; residual add + LN
    (stt-accum sums + ACT Square pass) with batched residual prefetch.
Next head pair's K/Q projections are interleaved into the attention loop as
single 8-matmul bursts (borrowing a scores-pool PSUM slot ahead of the
iteration's own scores so they ride the freed exp(j-1) bank), with
weight-column stripes prefetched ~10 iterations early as 8 per-c DMAs.
Startup interleaves ht[c]/wvh[c] DMA pairs across queues so the V-projection
c-chain starts consuming at ~4us and warms the PE clock early.
"""
import numpy as np
import ml_dtypes

N_HEAD, D_MODEL, D_HEAD = 16, 1024, 64
SEQ, BSZ = 2048, 4
QLEN = SEQ // 2
SCALE = 1.0 / D_HEAD ** 0.5
LN_EPS = 1e-5
P = 128
NSL = 512                   # matmul moving-operand slab (one PSUM bank fp32)
CT = D_MODEL // P           # 8 contraction tiles
ET = D_MODEL // P           # 8 e-tiles (2 heads each)
JT = SEQ // P               # 16 key tiles
JS = SEQ // NSL             # 4 key slabs
IS = QLEN // NSL            # 2 query slabs
TQ = QLEN // P              # 8 query-row tiles
HP = N_HEAD // 2            # 8 head pairs

_CACHE = {}


def _build():
    from contextlib import ExitStack
    import concourse.bass as bass
    import concourse.mybir as mybir
    import concourse.tile as tile
    from concourse import bacc

    dt = mybir.dt
    f32, bf16 = dt.float32, dt.bfloat16
    AF = mybir.ActivationFunctionType
    ALU = mybir.AluOpType

    nc = bacc.Bacc(None, target_bir_lowering=False)

    hT = nc.dram_tensor("hT", [D_MODEL, SEQ], bf16, kind="ExternalInput")
    hq = nc.dram_tensor("hq", [QLEN, D_MODEL], f32, kind="ExternalInput")
    wq = nc.dram_tensor("wq", [D_MODEL, D_MODEL], bf16, kind="ExternalInput")
    wk = nc.dram_tensor("wk", [D_MODEL, D_MODEL], bf16, kind="ExternalInput")
    wv = nc.dram_tensor("wv", [D_MODEL, D_MODEL], bf16, kind="ExternalInput")
    wo = nc.dram_tensor("wo", [D_MODEL, D_MODEL], bf16, kind="ExternalInput")
    mb = nc.dram_tensor("mb", [P, JT], f32, kind="ExternalInput")
    gam = nc.dram_tensor("gam", [D_MODEL], f32, kind="ExternalInput")
    bet = nc.dram_tensor("bet", [D_MODEL], f32, kind="ExternalInput")
    out = nc.dram_tensor("out", [QLEN, D_MODEL], f32, kind="ExternalOutput")

    with tile.TileContext(nc) as tc, ExitStack() as ctx:
        persist = ctx.enter_context(tc.tile_pool(name="persist", bufs=1))

        avt_sb = [persist.tile([P, QLEN], bf16, name=f"avt{e}") for e in range(ET)]
        mask_sb = persist.tile([P, JT], f32, name="mask")
        eps_sb = persist.tile([P, 1], f32, name="eps")

        nc.vector.memset(eps_sb, LN_EPS)

        nc.gpsimd.dma_start(out=mask_sb, in_=mb[0:P, :])

        # ---- phase-3 weights: load early into the region wvp freed ---------
        w3p = ctx.enter_context(tc.tile_pool(name="w3p", bufs=1))
        wo_sb = [w3p.tile([P, D_MODEL], bf16, name=f"wo{c}") for c in range(CT)]
        gam_sb = w3p.tile([P, D_MODEL], f32, name="gamr")
        bet_sb = w3p.tile([P, D_MODEL], f32, name="betr")

        # ---- phase 1 scope: h^T residency + streamed W columns + K/Q/V data
        # (all freed before the output-projection/LN phase)
        ph1_ctx = ExitStack()
        ph1 = ph1_ctx.enter_context(tc.tile_pool(name="ph1", bufs=1))
        ht_sb = [ph1.tile([P, SEQ], bf16, name=f"ht{c}") for c in range(CT)]
        kt_sb = [ph1.tile([P, SEQ], bf16, name=f"kt{e}") for e in range(ET)]
        qt_sb = [ph1.tile([P, QLEN], bf16, name=f"qt{e}") for e in range(ET)]
        # per head pair: [V_even | ones64 | V_odd]; the even head's stationary
        # window is cols 0:128 ([V|ones] -> av rows 0-63, den rows 64-127),
        # the odd head's is cols 64:192 ([ones|V] -> den rows 0-63, av rows
        # 64-127). One shared ones block per pair keeps SBUF in budget.
        v_sb = [ph1.tile([P, HP, 3, D_HEAD], bf16, name=f"v{t}")
                for t in range(JT)]
        for t in range(JT):
            nc.gpsimd.memset(v_sb[t][:, :, 1, :], 1.0)

        wcol = ph1_ctx.enter_context(tc.tile_pool(name="wcol", bufs=2))
        wcolq = ph1_ctx.enter_context(tc.tile_pool(name="wcolq", bufs=1))

        # one weight column-stripe as 8 contiguous per-c DMAs spread over
        # queues (a single strided dma_start here costs ~1024 descriptors on
        # one queue, ~38us serial)
        wcol_engs = [nc.sync, nc.scalar, nc.gpsimd]

        def load_wcol(w, e, tag, engs=None):
            engs = engs or wcol_engs
            pool = wcolq if tag == "wqc" else wcol
            wc = pool.tile([P, CT, P], bf16, tag=tag, name=f"{tag}{e}")
            for c in range(CT):
                engs[c % len(engs)].dma_start(
                    out=wc[:, c, :],
                    in_=w[c * P:(c + 1) * P, e * P:(e + 1) * P])
            return wc

        # mid-attention loads must stay off the Scalar (ACT) queue: a
        # dma_start there blocks the exp stream behind pool semaphores
        attn_engs = [nc.sync, nc.gpsimd]

        # startup DMA priority: the first K-proj matmul needs wkc(0) + ht
        # tiles first; wkc(1) is prefetched for head pair 0's interleaved
        # K-proj bursts.
        wc0 = load_wcol(wk, 0, "wkc")
        for c in range(CT):
            eng = nc.sync if c % 2 == 0 else nc.scalar
            eng.dma_start(out=ht_sb[c], in_=hT[c * P:(c + 1) * P, :])

        def kq_group(ps_ap, wc, moving, sl):
            """8 accumulating matmuls: one K/Q-proj output group into psum."""
            for c in range(CT):
                nc.tensor.matmul(ps_ap, wc[:, c, :],
                                 moving[c][:, sl * NSL:(sl + 1) * NSL],
                                 start=(c == 0), stop=(c == CT - 1))

        # prephase: K(0), Q(0), V (own pools, closed before attention)
        with tc.tile_pool(name="wvp", bufs=1) as wvp, \
             tc.tile_pool(name="psA", bufs=6, space="PSUM") as psA:
            wvh0 = [wvp.tile([P, NSL], bf16, tag=f"wvh{c}", name=f"wv0_{c}")
                    for c in range(CT)]
            wcq0 = load_wcol(wq, 0, "wqc")
            for c in range(CT):
                wcol_engs[c % 3].dma_start(out=wvh0[c],
                                           in_=wv[c * P:(c + 1) * P, 0:NSL])
            def v_half(es, wvh):
                for t in range(JT):
                    ps = psA.tile([P, NSL], f32, tag="psa", name=f"psv{t}_{es}")
                    for c in range(CT):
                        nc.tensor.matmul(ps, ht_sb[c][:, t * P:(t + 1) * P],
                                         wvh[c],
                                         start=(c == 0), stop=(c == CT - 1))
                    psr = ps[:, :].rearrange("p (pr par d) -> p pr par d",
                                             par=2, d=D_HEAD)
                    nc.vector.tensor_copy(
                        v_sb[t][:, es * 4:(es + 1) * 4, 0, :], psr[:, :, 0, :])
                    nc.vector.tensor_copy(
                        v_sb[t][:, es * 4:(es + 1) * 4, 2, :], psr[:, :, 1, :])

            # order: V(es=0), then K0/Q0 (fills the Wv second-half reload
            # gap with useful matmuls), then V(es=1)
            v_half(0, wvh0)
            wvh1 = [wvp.tile([P, NSL], bf16, tag=f"wvh{c}",
                             name=f"wv1_{c}") for c in range(CT)]
            for c in range(CT):
                wcol_engs[c % 3].dma_start(
                    out=wvh1[c], in_=wv[c * P:(c + 1) * P, NSL:2 * NSL])
            wc = wc0
            for j in range(JS):
                ps = psA.tile([P, NSL], f32, tag="psa", name=f"psk0_{j}")
                kq_group(ps, wc, ht_sb, j)
                nc.vector.tensor_copy(kt_sb[0][:, j * NSL:(j + 1) * NSL], ps)
            wc = wcq0
            for i in range(IS):
                ps = psA.tile([P, NSL], f32, tag="psa", name=f"psq0_{i}")
                kq_group(ps, wc, ht_sb, i)
                nc.vector.tensor_copy(qt_sb[0][:, i * NSL:(i + 1) * NSL], ps)
            v_half(1, wvh1)

        def emit_pv(nc, v_sb, avd, hp, j, pts):
            first, last = (j == 0), (j == JT - 1)
            for hb in range(2):
                stat = v_sb[j][:, hp, hb:hb + 2, :]
                for i in range(IS):
                    sl = slice(i * NSL, (i + 1) * NSL)
                    nc.tensor.matmul(avd[hb][i], stat,
                                     pts[hb][:, sl], start=first, stop=last)

        for c in range(CT):
            nc.scalar.dma_start(out=wo_sb[c], in_=wo[c * P:(c + 1) * P, :])
        nc.gpsimd.dma_start(out=gam_sb,
                            in_=bass.AP(tensor=gam, offset=0, ap=[[0, P], [1, D_MODEL]]))
        nc.gpsimd.dma_start(out=bet_sb,
                            in_=bass.AP(tensor=bet, offset=0, ap=[[0, P], [1, D_MODEL]]))

        # ---- attention ------------------------------------------------------
        attn_ctx = ExitStack()
        scp = attn_ctx.enter_context(tc.tile_pool(name="scp", bufs=2, space="PSUM"))
        avp = attn_ctx.enter_context(tc.tile_pool(name="avp", bufs=1, space="PSUM"))
        ptp = attn_ctx.enter_context(tc.tile_pool(name="ptp", bufs=7))
        nrm = attn_ctx.enter_context(tc.tile_pool(name="nrm", bufs=2))

        for hp in range(HP):
            avd = [[avp.tile([P, NSL], f32, tag=f"avd{hb}{i}",
                             name=f"avd{hp}_{hb}_{i}") for i in range(IS)]
                   for hb in range(2)]
            # interleaved projection work for the NEXT head pair, borrowing
            # scores-pool psum slots. One 8-matmul slab (~1.8us) per burst so
            # the ScalarE exp stream never runs dry; weight-column stripes
            # are prefetched ~10 iterations ahead of first use.
            if hp + 1 < HP:
                proj_work = {1: ("kl", 0), 3: ("k", 0), 5: ("k", 1),
                             7: ("k", 2), 9: ("k", 3), 10: ("ql", 0),
                             11: ("q", 0), 13: ("q", 1)}
            else:
                proj_work = {}
            prev_pt = None
            wc_k = None
            wc_q = None

            for j in range(JT):
                # interleaved projection work first: the borrow tile then
                # takes the psum slot freed by exp(j-1) instead of blocking
                # on this iteration's own exps, and the 8-matmul chain
                # overlaps the queued exp(j-1) backlog
                if j in proj_work:
                    kind, sl = proj_work[j]
                    if kind == "kl":
                        wc_k = load_wcol(wk, hp + 1, "wkc", attn_engs)
                    elif kind == "ql":
                        wc_q = load_wcol(wq, hp + 1, "wqc", attn_engs)
                    else:
                        borrow = scp.tile([P, QLEN], f32, tag="sc",
                                          name=f"bw{hp}_{j}")
                        wc, dst = (wc_k, kt_sb) if kind == "k" else (wc_q, qt_sb)
                        kq_group(borrow[:, 0:NSL], wc, ht_sb, sl)
                        nc.vector.tensor_copy(
                            dst[hp + 1][:, sl * NSL:(sl + 1) * NSL],
                            borrow[:, 0:NSL])
                scs = [scp.tile([P, QLEN], f32, tag="sc", name=f"sc{hp}_{j}_{hb}")
                       for hb in range(2)]
                for hb in range(2):
                    base = hb * 64
                    for i in range(IS):
                        nc.tensor.matmul(
                            scs[hb][:, i * NSL:(i + 1) * NSL],
                            kt_sb[hp][base:base + 64, j * P:(j + 1) * P],
                            qt_sb[hp][base:base + 64, i * NSL:(i + 1) * NSL],
                            start=True, stop=True, tile_position=(base, 0))
                cur_pt = []
                for hb in range(2):
                    pt_t = ptp.tile([P, QLEN], bf16, tag="pt",
                                    name=f"pt{hp}_{j}_{hb}")
                    nc.scalar.activation(pt_t, scs[hb], AF.Exp,
                                         bias=mask_sb[:, j:j + 1], scale=SCALE)
                    cur_pt.append(pt_t)

                if prev_pt is not None:
                    emit_pv(nc, v_sb, avd, hp, j - 1, prev_pt)
                prev_pt = cur_pt


            # last PV round
            emit_pv(nc, v_sb, avd, hp, JT - 1, prev_pt)

            # normalize: even head has av@0-63/den@64-127, odd head the
            # mirror. Reciprocal_approx_fast only works at base partition 0,
            # so route the den rows there (DVE cross-partition copies are
            # fine), then stt straight from psum into the bf16 avt strip.
            for hb in range(2):
                for i in range(IS):
                    sl = slice(i * NSL, (i + 1) * NSL)
                    dcp = nrm.tile([64, NSL], f32, tag="dcp",
                                   name=f"dcp{hp}_{hb}_{i}")
                    den_rows = avd[hb][i][64:P, :] if hb == 0 \
                        else avd[hb][i][0:64, :]
                    nc.vector.tensor_copy(dcp, den_rows)
                    rep = nrm.tile([64, NSL], f32, tag="rep",
                                   name=f"rep{hp}_{hb}_{i}")
                    nc.vector.reciprocal_approx_fast(out=rep, in_=dcp)
                    if hb == 0:
                        nc.vector.scalar_tensor_tensor(
                            out=avt_sb[hp][0:64, sl],
                            in0=avd[0][i][0:64, :], scalar=1.0,
                            in1=rep, op0=ALU.mult, op1=ALU.mult)
                    else:
                        rep64 = nrm.tile([P, NSL], f32, tag="rep64",
                                         name=f"rep64_{hp}_{i}")
                        nc.vector.tensor_copy(rep64[64:P, :], rep)
                        nc.vector.scalar_tensor_tensor(
                            out=avt_sb[hp][64:P, sl],
                            in0=avd[1][i][64:P, :], scalar=1.0,
                            in1=rep64[64:P, :], op0=ALU.mult, op1=ALU.mult)

        # ---- output projection + residual + layernorm -----------------------
        attn_ctx.close()
        ph1_ctx.close()

        pso = ctx.enter_context(tc.tile_pool(name="pso", bufs=8, space="PSUM"))
        lnp = ctx.enter_context(tc.tile_pool(name="lnp", bufs=3))
        lns = ctx.enter_context(tc.tile_pool(name="lns", bufs=8))
        hqp = ctx.enter_context(tc.tile_pool(name="hqp", bufs=1))

        # batch all residual loads upfront across queues so they overlap the
        # first output-projection matmuls instead of gating each tile
        hq_ts = [hqp.tile([P, D_MODEL], f32, name=f"hq{t}") for t in range(TQ)]
        for t in range(TQ):
            eng = (nc.sync, nc.scalar, nc.gpsimd)[t % 3]
            eng.dma_start(out=hq_ts[t], in_=hq[t * P:(t + 1) * P, :])

        for t in range(TQ):
            hq_t = hq_ts[t]
            xs = lnp.tile([P, D_MODEL], f32, tag="xs", name=f"xs{t}")
            sums = lns.tile([P, 2], f32, tag="sm", name=f"sm{t}")
            for m in range(2):
                ps = pso.tile([P, NSL], f32, tag="po", name=f"po{t}_{m}")
                for e in range(ET):
                    nc.tensor.matmul(ps, avt_sb[e][:, t * P:(t + 1) * P],
                                     wo_sb[e][:, m * NSL:(m + 1) * NSL],
                                     start=(e == 0), stop=(e == ET - 1))
                nc.vector.scalar_tensor_tensor(
                    out=xs[:, m * NSL:(m + 1) * NSL], in0=ps, scalar=1.0,
                    in1=hq_t[:, m * NSL:(m + 1) * NSL],
                    op0=ALU.mult, op1=ALU.add,
                    accum_out=sums[:, m:m + 1])
            # mean/var via accum sums + ACT Square pass; mean = (s0+s1)/D,
            # var = sq/D - mean^2
            sq = lns.tile([P, 2], f32, tag="sq", name=f"sq{t}")
            xsq = lnp.tile([P, D_MODEL], f32, tag="xq", name=f"xq{t}")
            for m in range(2):
                nc.scalar.activation(xsq[:, m * NSL:(m + 1) * NSL],
                                     xs[:, m * NSL:(m + 1) * NSL], AF.Square,
                                     accum_out=sq[:, m:m + 1])
            mean = lns.tile([P, 1], f32, tag="mn", name=f"mn{t}")
            nc.vector.tensor_add(mean, sums[:, 0:1], sums[:, 1:2])
            nc.vector.tensor_scalar_mul(mean, mean, 1.0 / D_MODEL)
            msq = lns.tile([P, 1], f32, tag="mq", name=f"mq{t}")
            nc.vector.tensor_mul(msq, mean, mean)
            var = lns.tile([P, 1], f32, tag="vr", name=f"vr{t}")
            nc.vector.tensor_add(var, sq[:, 0:1], sq[:, 1:2])
            nc.vector.scalar_tensor_tensor(
                out=var, in0=var, scalar=1.0 / D_MODEL, in1=msq,
                op0=ALU.mult, op1=ALU.subtract)
            std = lns.tile([P, 1], f32, tag="sd", name=f"sd{t}")
            nc.scalar.activation(std, var, AF.Sqrt, bias=eps_sb[:, 0:1])
            rstd = lns.tile([P, 1], f32, tag="rs", name=f"rs{t}")
            nc.vector.reciprocal_approx_fast(out=rstd, in_=std)
            nmr = lns.tile([P, 1], f32, tag="nm", name=f"nm{t}")
            nc.vector.tensor_scalar_mul(nmr, mean, -1.0)
            gs = lnp.tile([P, D_MODEL], f32, tag="gs", name=f"gs{t}")
            nc.vector.tensor_scalar(out=gs, in0=gam_sb,
                                    scalar1=rstd[:, 0:1], scalar2=None,
                                    op0=ALU.mult)
            xg = lnp.tile([P, D_MODEL], f32, tag="xg", name=f"xg{t}")
            nc.vector.scalar_tensor_tensor(
                out=xg, in0=xs, scalar=nmr[:, 0:1], in1=gs,
                op0=ALU.add, op1=ALU.mult)
            xn = lnp.tile([P, D_MODEL], f32, tag="xn", name=f"xn{t}")
            nc.gpsimd.tensor_add(xn, xg, bet_sb)
            nc.sync.dma_start(out=out[t * P:(t + 1) * P, :], in_=xn)

    nc.compile()
    return nc


def _get_nc():
    if "nc" not in _CACHE:
        _CACHE["nc"] = _build()
    return _CACHE["nc"]


def _make_in_maps(inputs):
    bf = ml_dtypes.bfloat16
    h = np.asarray(inputs["h"], dtype=np.float32)
    mask = np.asarray(inputs["attn_mask"])
    Wq = np.asarray(inputs["Wq"], dtype=np.float32)
    Wkv = np.asarray(inputs["Wkv"], dtype=np.float32)
    Wo = np.asarray(inputs["Wo"], dtype=np.float32)
    gamma = np.asarray(inputs["gamma"], dtype=np.float32)
    beta = np.asarray(inputs["beta"], dtype=np.float32)

    wq_b = np.ascontiguousarray(Wq.astype(bf))
    wk_b = np.ascontiguousarray(Wkv[:, :D_MODEL].astype(bf))
    wv_b = np.ascontiguousarray(Wkv[:, D_MODEL:].astype(bf))
    wo_b = np.ascontiguousarray(Wo.astype(bf))

    in_maps = []
    for c in range(8):
        b, half = divmod(c, 2)
        hb = h[:, b, :]
        hT_b = hb.T.astype(bf)
        own = slice(half * QLEN, (half + 1) * QLEN)
        other = slice((1 - half) * QLEN, (2 - half) * QLEN)
        # own query-half first: keys are in core-local order, so the Q
        # projection can read the first half of hT uniformly on every core.
        # The mask is reordered identically; attention is key-order-invariant.
        hT_r = np.ascontiguousarray(np.concatenate(
            [hT_b[:, own], hT_b[:, other]], axis=1))
        mb_full = np.where(mask[:, b], np.float32(-1e9), np.float32(0.0))
        in_maps.append({
            "hT": hT_r,
            "hq": np.ascontiguousarray(hb[own, :]),
            "wq": wq_b, "wk": wk_b, "wv": wv_b, "wo": wo_b,
            "mb": np.ascontiguousarray(
                np.concatenate([mb_full[own], mb_full[other]])
                .reshape(JT, P).T),
            "gam": gamma, "bet": beta,
        })
    return in_maps


def _run(in_maps, **kwargs):
    from concourse.bass_utils import run_bass_kernel_spmd
    return run_bass_kernel_spmd(_get_nc(), in_maps, core_ids=list(range(8)),
                                **kwargs)


def kernel(**inputs) -> np.ndarray:
    res = _run(_make_in_maps(inputs))
    out = np.empty((SEQ, BSZ, D_MODEL), dtype=np.float32)
    for c in range(8):
        b, half = divmod(c, 2)
        out[half * QLEN:(half + 1) * QLEN, :, :][:, b, :] = res.results[c]["out"]
    return out



# revision 48
# speedup vs baseline: 1.0311x; 1.0311x over previous
"""MultiHeadAttn (post-LN, key-padding mask) Trainium2 Bass kernel, 8 cores.

Problem: h [S=2048, B=4, D=1024] f32; 16 heads x 64; key-padding mask [S, B];
out = LayerNorm(h + MHA(h)).

Sharding: core c handles batch b = c//2 and query half qh = c%2 (1024 query
rows), with all 16 heads and the full 2048-key context for that batch.
KV projections are recomputed by both cores of a batch pair (no collectives).

Per-core device pipeline (all matmuls bf16, fp32 accumulation in PSUM):
  - K^T/Q^T proj: stationary Wk/Wq column tiles, moving h^T -> [e, t] layout.
  - V proj: stationary h^T tiles, moving Wv -> natural [t, e] layout, stored
    as [V_h | ones64] per head (128-wide stationary).
  - Attention per head pair: scores^T [j,i] via row-paired matmuls (two heads
    in row strips 0-63 / 64-127 of the PE array), exp via ScalarE with the
    key-padding bias as a per-partition bias and 1/sqrt(dh) as the scale,
    PV with the [V|ones] stationary (M=128: rows 0-63 attn_vec, rows 64-127
    the softmax denominator replicated), then normalize via cross-partition
    copy + reciprocal_approx_fast + one stt per head.
  - Output proj: stationary attn_vec^T tiles, moving Wo; residual add + LN
    (stt-accum sums + ACT Square pass) with batched residual prefetch.
Next head pair's K/Q projections are interleaved into the attention loop as
single 8-matmul bursts (borrowing a scores-pool PSUM slot ahead of the
iteration's own scores so they ride the freed exp(j-1) bank), with
weight-column stripes prefetched ~10 iterations early as 8 per-c DMAs.
Startup interleaves ht[c]/wvh[c] DMA pairs across queues so the V-projection
c-chain starts consuming at ~4us and warms the PE clock early.
"""
import numpy as np
import ml_dtypes

N_HEAD, D_MODEL, D_HEAD = 16, 1024, 64
SEQ, BSZ = 2048, 4
QLEN = SEQ // 2
SCALE = 1.0 / D_HEAD ** 0.5
LN_EPS = 1e-5
P = 128
NSL = 512                   # matmul moving-operand slab (one PSUM bank fp32)
CT = D_MODEL // P           # 8 contraction tiles
ET = D_MODEL // P           # 8 e-tiles (2 heads each)
JT = SEQ // P               # 16 key tiles
JS = SEQ // NSL             # 4 key slabs
IS = QLEN // NSL            # 2 query slabs
TQ = QLEN // P              # 8 query-row tiles
HP = N_HEAD // 2            # 8 head pairs

_CACHE = {}


def _build():
    from contextlib import ExitStack
    import concourse.bass as bass
    import concourse.mybir as mybir
    import concourse.tile as tile
    from concourse import bacc

    dt = mybir.dt
    f32, bf16 = dt.float32, dt.bfloat16
    AF = mybir.ActivationFunctionType
    ALU = mybir.AluOpType

    nc = bacc.Bacc(None, target_bir_lowering=False)

    hT = nc.dram_tensor("hT", [D_MODEL, SEQ], bf16, kind="ExternalInput")
    hq = nc.dram_tensor("hq", [QLEN, D_MODEL], f32, kind="ExternalInput")
    wq = nc.dram_tensor("wq", [D_MODEL, D_MODEL], bf16, kind="ExternalInput")
    wk = nc.dram_tensor("wk", [D_MODEL, D_MODEL], bf16, kind="ExternalInput")
    wv = nc.dram_tensor("wv", [D_MODEL, D_MODEL], bf16, kind="ExternalInput")
    wo = nc.dram_tensor("wo", [D_MODEL, D_MODEL], bf16, kind="ExternalInput")
    mb = nc.dram_tensor("mb", [P, JT], f32, kind="ExternalInput")
    gam = nc.dram_tensor("gam", [D_MODEL], f32, kind="ExternalInput")
    bet = nc.dram_tensor("bet", [D_MODEL], f32, kind="ExternalInput")
    out = nc.dram_tensor("out", [QLEN, D_MODEL], f32, kind="ExternalOutput")

    with tile.TileContext(nc) as tc, ExitStack() as ctx:
        persist = ctx.enter_context(tc.tile_pool(name="persist", bufs=1))

        avt_sb = [persist.tile([P, QLEN], bf16, name=f"avt{e}") for e in range(ET)]
        mask_sb = persist.tile([P, JT], f32, name="mask")
        eps_sb = persist.tile([P, 1], f32, name="eps")

        nc.vector.memset(eps_sb, LN_EPS)

        nc.gpsimd.dma_start(out=mask_sb, in_=mb[0:P, :])

        # ---- phase-3 weights: load early into the region wvp freed ---------
        w3p = ctx.enter_context(tc.tile_pool(name="w3p", bufs=1))
        wo_sb = [w3p.tile([P, D_MODEL], bf16, name=f"wo{c}") for c in range(CT)]
        gam_sb = w3p.tile([P, D_MODEL], f32, name="gamr")
        bet_sb = w3p.tile([P, D_MODEL], f32, name="betr")

        # ---- phase 1 scope: h^T residency + streamed W columns + K/Q/V data
        # (all freed before the output-projection/LN phase)
        ph1_ctx = ExitStack()
        ph1 = ph1_ctx.enter_context(tc.tile_pool(name="ph1", bufs=1))
        ht_sb = [ph1.tile([P, SEQ], bf16, name=f"ht{c}") for c in range(CT)]
        kt_sb = [ph1.tile([P, SEQ], bf16, name=f"kt{e}") for e in range(ET)]
        qt_sb = [ph1.tile([P, QLEN], bf16, name=f"qt{e}") for e in range(ET)]
        # per head pair: [V_even | ones64 | V_odd]; the even head's stationary
        # window is cols 0:128 ([V|ones] -> av rows 0-63, den rows 64-127),
        # the odd head's is cols 64:192 ([ones|V] -> den rows 0-63, av rows
        # 64-127). One shared ones block per pair keeps SBUF in budget.
        v_sb = [ph1.tile([P, HP, 3, D_HEAD], bf16, name=f"v{t}")
                for t in range(JT)]
        for t in range(JT):
            nc.gpsimd.memset(v_sb[t][:, :, 1, :], 1.0)

        wcol = ph1_ctx.enter_context(tc.tile_pool(name="wcol", bufs=2))
        wcolq = ph1_ctx.enter_context(tc.tile_pool(name="wcolq", bufs=1))

        # one weight column-stripe as 8 contiguous per-c DMAs spread over
        # queues (a single strided dma_start here costs ~1024 descriptors on
        # one queue, ~38us serial)
        wcol_engs = [nc.sync, nc.scalar, nc.gpsimd]

        def load_wcol(w, e, tag, engs=None):
            engs = engs or wcol_engs
            pool = wcolq if tag == "wqc" else wcol
            wc = pool.tile([P, CT, P], bf16, tag=tag, name=f"{tag}{e}")
            for c in range(CT):
                engs[c % len(engs)].dma_start(
                    out=wc[:, c, :],
                    in_=w[c * P:(c + 1) * P, e * P:(e + 1) * P])
            return wc

        # mid-attention loads must stay off the Scalar (ACT) queue: a
        # dma_start there blocks the exp stream behind pool semaphores
        attn_engs = [nc.sync, nc.gpsimd]

        # startup DMA priority: V proj runs first and its c-chain consumes
        # ht[c]+wvh0[c] in order, so interleave those pairs across queues --
        # the PE then starts at ~4us (and warms its clock) instead of
        # waiting for the full h^T load.

        def kq_group(ps_ap, wc, moving, sl):
            """8 accumulating matmuls: one K/Q-proj output group into psum."""
            for c in range(CT):
                nc.tensor.matmul(ps_ap, wc[:, c, :],
                                 moving[c][:, sl * NSL:(sl + 1) * NSL],
                                 start=(c == 0), stop=(c == CT - 1))

        # prephase: K(0), Q(0), V (own pools, closed before attention)
        with tc.tile_pool(name="wvp", bufs=1) as wvp, \
             tc.tile_pool(name="psA", bufs=6, space="PSUM") as psA:
            wvh0 = [wvp.tile([P, NSL], bf16, tag=f"wvh{c}", name=f"wv0_{c}")
                    for c in range(CT)]
            for c in range(CT):
                eng = wcol_engs[c % 3]
                eng.dma_start(out=ht_sb[c], in_=hT[c * P:(c + 1) * P, :])
                eng.dma_start(out=wvh0[c],
                              in_=wv[c * P:(c + 1) * P, 0:NSL])
            wc0 = load_wcol(wk, 0, "wkc")
            wcq0 = load_wcol(wq, 0, "wqc")
            def v_half(es, wvh):
                for t in range(JT):
                    ps = psA.tile([P, NSL], f32, tag="psa", name=f"psv{t}_{es}")
                    for c in range(CT):
                        nc.tensor.matmul(ps, ht_sb[c][:, t * P:(t + 1) * P],
                                         wvh[c],
                                         start=(c == 0), stop=(c == CT - 1))
                    psr = ps[:, :].rearrange("p (pr par d) -> p pr par d",
                                             par=2, d=D_HEAD)
                    nc.vector.tensor_copy(
                        v_sb[t][:, es * 4:(es + 1) * 4, 0, :], psr[:, :, 0, :])
                    nc.vector.tensor_copy(
                        v_sb[t][:, es * 4:(es + 1) * 4, 2, :], psr[:, :, 1, :])

            # order: V(es=0), then K0/Q0 (fills the Wv second-half reload
            # gap with useful matmuls), then V(es=1)
            v_half(0, wvh0)
            wvh1 = [wvp.tile([P, NSL], bf16, tag=f"wvh{c}",
                             name=f"wv1_{c}") for c in range(CT)]
            for c in range(CT):
                wcol_engs[c % 3].dma_start(
                    out=wvh1[c], in_=wv[c * P:(c + 1) * P, NSL:2 * NSL])
            wc = wc0
            for j in range(JS):
                ps = psA.tile([P, NSL], f32, tag="psa", name=f"psk0_{j}")
                kq_group(ps, wc, ht_sb, j)
                nc.vector.tensor_copy(kt_sb[0][:, j * NSL:(j + 1) * NSL], ps)
            wc = wcq0
            for i in range(IS):
                ps = psA.tile([P, NSL], f32, tag="psa", name=f"psq0_{i}")
                kq_group(ps, wc, ht_sb, i)
                nc.vector.tensor_copy(qt_sb[0][:, i * NSL:(i + 1) * NSL], ps)
            v_half(1, wvh1)

        def emit_pv(nc, v_sb, avd, hp, j, pts):
            first, last = (j == 0), (j == JT - 1)
            for hb in range(2):
                stat = v_sb[j][:, hp, hb:hb + 2, :]
                for i in range(IS):
                    sl = slice(i * NSL, (i + 1) * NSL)
                    nc.tensor.matmul(avd[hb][i], stat,
                                     pts[hb][:, sl], start=first, stop=last)

        for c in range(CT):
            nc.scalar.dma_start(out=wo_sb[c], in_=wo[c * P:(c + 1) * P, :])
        nc.gpsimd.dma_start(out=gam_sb,
                            in_=bass.AP(tensor=gam, offset=0, ap=[[0, P], [1, D_MODEL]]))
        nc.gpsimd.dma_start(out=bet_sb,
                            in_=bass.AP(tensor=bet, offset=0, ap=[[0, P], [1, D_MODEL]]))

        # ---- attention ------------------------------------------------------
        attn_ctx = ExitStack()
        scp = attn_ctx.enter_context(tc.tile_pool(name="scp", bufs=2, space="PSUM"))
        avp = attn_ctx.enter_context(tc.tile_pool(name="avp", bufs=1, space="PSUM"))
        ptp = attn_ctx.enter_context(tc.tile_pool(name="ptp", bufs=7))
        nrm = attn_ctx.enter_context(tc.tile_pool(name="nrm", bufs=2))

        for hp in range(HP):
            avd = [[avp.tile([P, NSL], f32, tag=f"avd{hb}{i}",
                             name=f"avd{hp}_{hb}_{i}") for i in range(IS)]
                   for hb in range(2)]
            # interleaved projection work for the NEXT head pair, borrowing
            # scores-pool psum slots. One 8-matmul slab (~1.8us) per burst so
            # the ScalarE exp stream never runs dry; weight-column stripes
            # are prefetched ~10 iterations ahead of first use.
            if hp + 1 < HP:
                proj_work = {1: ("kl", 0), 3: ("k", 0), 5: ("k", 1),
                             7: ("k", 2), 9: ("k", 3), 10: ("ql", 0),
                             11: ("q", 0), 13: ("q", 1)}
            else:
                proj_work = {}
            prev_pt = None
            wc_k = None
            wc_q = None

            for j in range(JT):
                # interleaved projection work first: the borrow tile then
                # takes the psum slot freed by exp(j-1) instead of blocking
                # on this iteration's own exps, and the 8-matmul chain
                # overlaps the queued exp(j-1) backlog
                if j in proj_work:
                    kind, sl = proj_work[j]
                    if kind == "kl":
                        wc_k = load_wcol(wk, hp + 1, "wkc", attn_engs)
                    elif kind == "ql":
                        wc_q = load_wcol(wq, hp + 1, "wqc", attn_engs)
                    else:
                        borrow = scp.tile([P, QLEN], f32, tag="sc",
                                          name=f"bw{hp}_{j}")
                        wc, dst = (wc_k, kt_sb) if kind == "k" else (wc_q, qt_sb)
                        kq_group(borrow[:, 0:NSL], wc, ht_sb, sl)
                        nc.vector.tensor_copy(
                            dst[hp + 1][:, sl * NSL:(sl + 1) * NSL],
                            borrow[:, 0:NSL])
                scs = [scp.tile([P, QLEN], f32, tag="sc", name=f"sc{hp}_{j}_{hb}")
                       for hb in range(2)]
                for hb in range(2):
                    base = hb * 64
                    for i in range(IS):
                        nc.tensor.matmul(
                            scs[hb][:, i * NSL:(i + 1) * NSL],
                            kt_sb[hp][base:base + 64, j * P:(j + 1) * P],
                            qt_sb[hp][base:base + 64, i * NSL:(i + 1) * NSL],
                            start=True, stop=True, tile_position=(base, 0))
                cur_pt = []
                for hb in range(2):
                    pt_t = ptp.tile([P, QLEN], bf16, tag="pt",
                                    name=f"pt{hp}_{j}_{hb}")
                    nc.scalar.activation(pt_t, scs[hb], AF.Exp,
                                         bias=mask_sb[:, j:j + 1], scale=SCALE)
                    cur_pt.append(pt_t)

                if prev_pt is not None:
                    emit_pv(nc, v_sb, avd, hp, j - 1, prev_pt)
                prev_pt = cur_pt


            # last PV round
            emit_pv(nc, v_sb, avd, hp, JT - 1, prev_pt)

            # normalize: even head has av@0-63/den@64-127, odd head the
            # mirror. Reciprocal_approx_fast only works at base partition 0,
            # so route the den rows there (DVE cross-partition copies are
            # fine), then stt straight from psum into the bf16 avt strip.
            for hb in range(2):
                for i in range(IS):
                    sl = slice(i * NSL, (i + 1) * NSL)
                    dcp = nrm.tile([64, NSL], f32, tag="dcp",
                                   name=f"dcp{hp}_{hb}_{i}")
                    den_rows = avd[hb][i][64:P, :] if hb == 0 \
                        else avd[hb][i][0:64, :]
                    nc.vector.tensor_copy(dcp, den_rows)
                    rep = nrm.tile([64, NSL], f32, tag="rep",
                                   name=f"rep{hp}_{hb}_{i}")
                    nc.vector.reciprocal_approx_fast(out=rep, in_=dcp)
                    if hb == 0:
                        nc.vector.scalar_tensor_tensor(
                            out=avt_sb[hp][0:64, sl],
                            in0=avd[0][i][0:64, :], scalar=1.0,
                            in1=rep, op0=ALU.mult, op1=ALU.mult)
                    else:
                        rep64 = nrm.tile([P, NSL], f32, tag="rep64",
                                         name=f"rep64_{hp}_{i}")
                        nc.vector.tensor_copy(rep64[64:P, :], rep)
                        nc.vector.scalar_tensor_tensor(
                            out=avt_sb[hp][64:P, sl],
                            in0=avd[1][i][64:P, :], scalar=1.0,
                            in1=rep64[64:P, :], op0=ALU.mult, op1=ALU.mult)

        # ---- output projection + residual + layernorm -----------------------
        attn_ctx.close()
        ph1_ctx.close()

        pso = ctx.enter_context(tc.tile_pool(name="pso", bufs=8, space="PSUM"))
        lnp = ctx.enter_context(tc.tile_pool(name="lnp", bufs=3))
        lns = ctx.enter_context(tc.tile_pool(name="lns", bufs=8))
        hqp = ctx.enter_context(tc.tile_pool(name="hqp", bufs=1))

        # batch all residual loads upfront across queues so they overlap the
        # first output-projection matmuls instead of gating each tile
        hq_ts = [hqp.tile([P, D_MODEL], f32, name=f"hq{t}") for t in range(TQ)]
        for t in range(TQ):
            eng = (nc.sync, nc.scalar, nc.gpsimd)[t % 3]
            eng.dma_start(out=hq_ts[t], in_=hq[t * P:(t + 1) * P, :])

        for t in range(TQ):
            hq_t = hq_ts[t]
            xs = lnp.tile([P, D_MODEL], f32, tag="xs", name=f"xs{t}")
            sums = lns.tile([P, 2], f32, tag="sm", name=f"sm{t}")
            for m in range(2):
                ps = pso.tile([P, NSL], f32, tag="po", name=f"po{t}_{m}")
                for e in range(ET):
                    nc.tensor.matmul(ps, avt_sb[e][:, t * P:(t + 1) * P],
                                     wo_sb[e][:, m * NSL:(m + 1) * NSL],
                                     start=(e == 0), stop=(e == ET - 1))
                nc.vector.scalar_tensor_tensor(
                    out=xs[:, m * NSL:(m + 1) * NSL], in0=ps, scalar=1.0,
                    in1=hq_t[:, m * NSL:(m + 1) * NSL],
                    op0=ALU.mult, op1=ALU.add,
                    accum_out=sums[:, m:m + 1])
            # mean/var via accum sums + ACT Square pass; mean = (s0+s1)/D,
            # var = sq/D - mean^2
            sq = lns.tile([P, 2], f32, tag="sq", name=f"sq{t}")
            xsq = lnp.tile([P, D_MODEL], f32, tag="xq", name=f"xq{t}")
            for m in range(2):
                nc.scalar.activation(xsq[:, m * NSL:(m + 1) * NSL],
                                     xs[:, m * NSL:(m + 1) * NSL], AF.Square,
                                     accum_out=sq[:, m:m + 1])
            mean = lns.tile([P, 1], f32, tag="mn", name=f"mn{t}")
            nc.vector.tensor_add(mean, sums[:, 0:1], sums[:, 1:2])
            nc.vector.tensor_scalar_mul(mean, mean, 1.0 / D_MODEL)
            msq = lns.tile([P, 1], f32, tag="mq", name=f"mq{t}")
            nc.vector.tensor_mul(msq, mean, mean)
            var = lns.tile([P, 1], f32, tag="vr", name=f"vr{t}")
            nc.vector.tensor_add(var, sq[:, 0:1], sq[:, 1:2])
            nc.vector.scalar_tensor_tensor(
                out=var, in0=var, scalar=1.0 / D_MODEL, in1=msq,
                op0=ALU.mult, op1=ALU.subtract)
            std = lns.tile([P, 1], f32, tag="sd", name=f"sd{t}")
            nc.scalar.activation(std, var, AF.Sqrt, bias=eps_sb[:, 0:1])
            rstd = lns.tile([P, 1], f32, tag="rs", name=f"rs{t}")
            nc.vector.reciprocal_approx_fast(out=rstd, in_=std)
            nmr = lns.tile([P, 1], f32, tag="nm", name=f"nm{t}")
            nc.vector.tensor_scalar_mul(nmr, mean, -1.0)
            gs = lnp.tile([P, D_MODEL], f32, tag="gs", name=f"gs{t}")
            nc.vector.tensor_scalar(out=gs, in0=gam_sb,
                                    scalar1=rstd[:, 0:1], scalar2=None,
                                    op0=ALU.mult)
            xg = lnp.tile([P, D_MODEL], f32, tag="xg", name=f"xg{t}")
            nc.vector.scalar_tensor_tensor(
                out=xg, in0=xs, scalar=nmr[:, 0:1], in1=gs,
                op0=ALU.add, op1=ALU.mult)
            xn = lnp.tile([P, D_MODEL], f32, tag="xn", name=f"xn{t}")
            nc.vector.tensor_add(xn, xg, bet_sb)
            nc.sync.dma_start(out=out[t * P:(t + 1) * P, :], in_=xn)

    nc.compile()
    return nc


def _get_nc():
    if "nc" not in _CACHE:
        _CACHE["nc"] = _build()
    return _CACHE["nc"]


def _make_in_maps(inputs):
    bf = ml_dtypes.bfloat16
    h = np.asarray(inputs["h"], dtype=np.float32)
    mask = np.asarray(inputs["attn_mask"])
    Wq = np.asarray(inputs["Wq"], dtype=np.float32)
    Wkv = np.asarray(inputs["Wkv"], dtype=np.float32)
    Wo = np.asarray(inputs["Wo"], dtype=np.float32)
    gamma = np.asarray(inputs["gamma"], dtype=np.float32)
    beta = np.asarray(inputs["beta"], dtype=np.float32)

    wq_b = np.ascontiguousarray(Wq.astype(bf))
    wk_b = np.ascontiguousarray(Wkv[:, :D_MODEL].astype(bf))
    wv_b = np.ascontiguousarray(Wkv[:, D_MODEL:].astype(bf))
    wo_b = np.ascontiguousarray(Wo.astype(bf))

    in_maps = []
    for c in range(8):
        b, half = divmod(c, 2)
        hb = h[:, b, :]
        hT_b = hb.T.astype(bf)
        own = slice(half * QLEN, (half + 1) * QLEN)
        other = slice((1 - half) * QLEN, (2 - half) * QLEN)
        # own query-half first: keys are in core-local order, so the Q
        # projection can read the first half of hT uniformly on every core.
        # The mask is reordered identically; attention is key-order-invariant.
        hT_r = np.ascontiguousarray(np.concatenate(
            [hT_b[:, own], hT_b[:, other]], axis=1))
        mb_full = np.where(mask[:, b], np.float32(-1e9), np.float32(0.0))
        in_maps.append({
            "hT": hT_r,
            "hq": np.ascontiguousarray(hb[own, :]),
            "wq": wq_b, "wk": wk_b, "wv": wv_b, "wo": wo_b,
            "mb": np.ascontiguousarray(
                np.concatenate([mb_full[own], mb_full[other]])
                .reshape(JT, P).T),
            "gam": gamma, "bet": beta,
        })
    return in_maps


def _run(in_maps, **kwargs):
    from concourse.bass_utils import run_bass_kernel_spmd
    return run_bass_kernel_spmd(_get_nc(), in_maps, core_ids=list(range(8)),
                                **kwargs)


def kernel(**inputs) -> np.ndarray:
    res = _run(_make_in_maps(inputs))
    out = np.empty((SEQ, BSZ, D_MODEL), dtype=np.float32)
    for c in range(8):
        b, half = divmod(c, 2)
        out[half * QLEN:(half + 1) * QLEN, :, :][:, b, :] = res.results[c]["out"]
    return out



# revision 50
# speedup vs baseline: 1.0314x; 1.0002x over previous
"""MultiHeadAttn (post-LN, key-padding mask) Trainium2 Bass kernel, 8 cores.

Problem: h [S=2048, B=4, D=1024] f32; 16 heads x 64; key-padding mask [S, B];
out = LayerNorm(h + MHA(h)).

Sharding: core c handles batch b = c//2 and query half qh = c%2 (1024 query
rows), with all 16 heads and the full 2048-key context for that batch.
KV projections are recomputed by both cores of a batch pair (no collectives).

Per-core device pipeline (all matmuls bf16, fp32 accumulation in PSUM):
  - K^T/Q^T proj: stationary Wk/Wq column tiles, moving h^T -> [e, t] layout.
  - V proj: stationary h^T tiles, moving Wv -> natural [t, e] layout, stored
    as [V_h | ones64] per head (128-wide stationary).
  - Attention per head pair: scores^T [j,i] via row-paired matmuls (two heads
    in row strips 0-63 / 64-127 of the PE array), exp via ScalarE with the
    key-padding bias as a per-partition bias and 1/sqrt(dh) as the scale,
    PV with the [V|ones] stationary (M=128: rows 0-63 attn_vec, rows 64-127
    the softmax denominator replicated), then normalize via cross-partition
    copy + reciprocal_approx_fast + one stt per head.
  - Output proj: stationary attn_vec^T tiles, moving Wo; residual add + LN
    (stt-accum sums + ACT Square pass) with batched residual prefetch.
Next head pair's K/Q projections are interleaved into the attention loop as
single 8-matmul bursts (borrowing a scores-pool PSUM slot ahead of the
iteration's own scores so they ride the freed exp(j-1) bank), with
weight-column stripes prefetched ~10 iterations early as 8 per-c DMAs.
Startup interleaves ht[c]/wvh[c] DMA pairs across queues so the V-projection
c-chain starts consuming at ~4us and warms the PE clock early.
"""
import numpy as np
import ml_dtypes

N_HEAD, D_MODEL, D_HEAD = 16, 1024, 64
SEQ, BSZ = 2048, 4
QLEN = SEQ // 2
SCALE = 1.0 / D_HEAD ** 0.5
LN_EPS = 1e-5
P = 128
NSL = 512                   # matmul moving-operand slab (one PSUM bank fp32)
CT = D_MODEL // P           # 8 contraction tiles
ET = D_MODEL // P           # 8 e-tiles (2 heads each)
JT = SEQ // P               # 16 key tiles
JS = SEQ // NSL             # 4 key slabs
IS = QLEN // NSL            # 2 query slabs
TQ = QLEN // P              # 8 query-row tiles
HP = N_HEAD // 2            # 8 head pairs

_CACHE = {}


def _build():
    from contextlib import ExitStack
    import concourse.bass as bass
    import concourse.mybir as mybir
    import concourse.tile as tile
    from concourse import bacc

    dt = mybir.dt
    f32, bf16 = dt.float32, dt.bfloat16
    AF = mybir.ActivationFunctionType
    ALU = mybir.AluOpType

    nc = bacc.Bacc(None, target_bir_lowering=False)

    hT = nc.dram_tensor("hT", [D_MODEL, SEQ], bf16, kind="ExternalInput")
    hq = nc.dram_tensor("hq", [QLEN, D_MODEL], f32, kind="ExternalInput")
    wq = nc.dram_tensor("wq", [D_MODEL, D_MODEL], bf16, kind="ExternalInput")
    wk = nc.dram_tensor("wk", [D_MODEL, D_MODEL], bf16, kind="ExternalInput")
    wv = nc.dram_tensor("wv", [D_MODEL, D_MODEL], bf16, kind="ExternalInput")
    wo = nc.dram_tensor("wo", [D_MODEL, D_MODEL], bf16, kind="ExternalInput")
    mb = nc.dram_tensor("mb", [P, JT], f32, kind="ExternalInput")
    gam = nc.dram_tensor("gam", [D_MODEL], f32, kind="ExternalInput")
    bet = nc.dram_tensor("bet", [D_MODEL], f32, kind="ExternalInput")
    out = nc.dram_tensor("out", [QLEN, D_MODEL], f32, kind="ExternalOutput")

    with tile.TileContext(nc) as tc, ExitStack() as ctx:
        persist = ctx.enter_context(tc.tile_pool(name="persist", bufs=1))

        avt_sb = [persist.tile([P, QLEN], bf16, name=f"avt{e}") for e in range(ET)]
        mask_sb = persist.tile([P, JT], f32, name="mask")
        eps_sb = persist.tile([P, 1], f32, name="eps")

        nc.vector.memset(eps_sb, LN_EPS)

        nc.gpsimd.dma_start(out=mask_sb, in_=mb[0:P, :])

        # ---- phase-3 weights: load early into the region wvp freed ---------
        w3p = ctx.enter_context(tc.tile_pool(name="w3p", bufs=1))
        wo_sb = [w3p.tile([P, D_MODEL], bf16, name=f"wo{c}") for c in range(CT)]
        gam_sb = w3p.tile([P, D_MODEL], f32, name="gamr")
        bet_sb = w3p.tile([P, D_MODEL], f32, name="betr")

        # ---- phase 1 scope: h^T residency + streamed W columns + K/Q/V data
        # (all freed before the output-projection/LN phase)
        ph1_ctx = ExitStack()
        ph1 = ph1_ctx.enter_context(tc.tile_pool(name="ph1", bufs=1))
        # two column-half tiles per c so the V-proj/K0 chains can start
        # after half the h^T load (dependencies are tile-granular)
        ht_sb = [[ph1.tile([P, QLEN], bf16, name=f"ht{c}_{h}")
                  for h in range(2)] for c in range(CT)]
        kt_sb = [ph1.tile([P, SEQ], bf16, name=f"kt{e}") for e in range(ET)]
        qt_sb = [ph1.tile([P, QLEN], bf16, name=f"qt{e}") for e in range(ET)]
        # per head pair: [V_even | ones64 | V_odd]; the even head's stationary
        # window is cols 0:128 ([V|ones] -> av rows 0-63, den rows 64-127),
        # the odd head's is cols 64:192 ([ones|V] -> den rows 0-63, av rows
        # 64-127). One shared ones block per pair keeps SBUF in budget.
        v_sb = [ph1.tile([P, HP, 3, D_HEAD], bf16, name=f"v{t}")
                for t in range(JT)]
        for t in range(JT):
            nc.gpsimd.memset(v_sb[t][:, :, 1, :], 1.0)

        wcol = ph1_ctx.enter_context(tc.tile_pool(name="wcol", bufs=2))
        wcolq = ph1_ctx.enter_context(tc.tile_pool(name="wcolq", bufs=1))

        # one weight column-stripe as 8 contiguous per-c DMAs spread over
        # queues (a single strided dma_start here costs ~1024 descriptors on
        # one queue, ~38us serial)
        wcol_engs = [nc.sync, nc.scalar, nc.gpsimd]

        def load_wcol(w, e, tag, engs=None):
            engs = engs or wcol_engs
            pool = wcolq if tag == "wqc" else wcol
            wc = pool.tile([P, CT, P], bf16, tag=tag, name=f"{tag}{e}")
            for c in range(CT):
                engs[c % len(engs)].dma_start(
                    out=wc[:, c, :],
                    in_=w[c * P:(c + 1) * P, e * P:(e + 1) * P])
            return wc

        # mid-attention loads must stay off the Scalar (ACT) queue: a
        # dma_start there blocks the exp stream behind pool semaphores
        attn_engs = [nc.sync, nc.gpsimd]

        # startup DMA priority: V proj runs first and its c-chain consumes
        # ht[c]+wvh0[c] in order, so interleave those pairs across queues --
        # the PE then starts at ~4us (and warms its clock) instead of
        # waiting for the full h^T load.

        def kq_group(ps_ap, wc, moving, sl):
            """8 accumulating matmuls: one K/Q-proj output group into psum."""
            half, hsl = divmod(sl, IS)
            for c in range(CT):
                nc.tensor.matmul(ps_ap, wc[:, c, :],
                                 moving[c][half][:, hsl * NSL:(hsl + 1) * NSL],
                                 start=(c == 0), stop=(c == CT - 1))

        # prephase: K(0), Q(0), V (own pools, closed before attention)
        with tc.tile_pool(name="wvp", bufs=1) as wvp, \
             tc.tile_pool(name="psA", bufs=6, space="PSUM") as psA:
            wvh0 = [wvp.tile([P, NSL], bf16, tag=f"wvh{c}", name=f"wv0_{c}")
                    for c in range(CT)]
            for c in range(CT):
                eng = wcol_engs[c % 3]
                eng.dma_start(out=ht_sb[c][0],
                              in_=hT[c * P:(c + 1) * P, 0:QLEN])
                eng.dma_start(out=wvh0[c],
                              in_=wv[c * P:(c + 1) * P, 0:NSL])
            for c in range(CT):
                wcol_engs[c % 3].dma_start(
                    out=ht_sb[c][1],
                    in_=hT[c * P:(c + 1) * P, QLEN:SEQ])
            wc0 = load_wcol(wk, 0, "wkc")
            wcq0 = load_wcol(wq, 0, "wqc")
            def v_half(es, wvh):
                for t in range(JT):
                    ps = psA.tile([P, NSL], f32, tag="psa", name=f"psv{t}_{es}")
                    for c in range(CT):
                        nc.tensor.matmul(ps, ht_sb[c][t // 8][:, (t % 8) * P:(t % 8 + 1) * P],
                                         wvh[c],
                                         start=(c == 0), stop=(c == CT - 1))
                    psr = ps[:, :].rearrange("p (pr par d) -> p pr par d",
                                             par=2, d=D_HEAD)
                    nc.vector.tensor_copy(
                        v_sb[t][:, es * 4:(es + 1) * 4, 0, :], psr[:, :, 0, :])
                    nc.vector.tensor_copy(
                        v_sb[t][:, es * 4:(es + 1) * 4, 2, :], psr[:, :, 1, :])

            # order: V(es=0), then K0/Q0 (fills the Wv second-half reload
            # gap with useful matmuls), then V(es=1)
            v_half(0, wvh0)
            wvh1 = [wvp.tile([P, NSL], bf16, tag=f"wvh{c}",
                             name=f"wv1_{c}") for c in range(CT)]
            for c in range(CT):
                wcol_engs[c % 3].dma_start(
                    out=wvh1[c], in_=wv[c * P:(c + 1) * P, NSL:2 * NSL])
            wc = wc0
            for j in range(JS):
                ps = psA.tile([P, NSL], f32, tag="psa", name=f"psk0_{j}")
                kq_group(ps, wc, ht_sb, j)
                nc.vector.tensor_copy(kt_sb[0][:, j * NSL:(j + 1) * NSL], ps)
            wc = wcq0
            for i in range(IS):
                ps = psA.tile([P, NSL], f32, tag="psa", name=f"psq0_{i}")
                kq_group(ps, wc, ht_sb, i)
                nc.vector.tensor_copy(qt_sb[0][:, i * NSL:(i + 1) * NSL], ps)
            v_half(1, wvh1)

        def emit_pv(nc, v_sb, avd, hp, j, pts):
            first, last = (j == 0), (j == JT - 1)
            for hb in range(2):
                stat = v_sb[j][:, hp, hb:hb + 2, :]
                for i in range(IS):
                    sl = slice(i * NSL, (i + 1) * NSL)
                    nc.tensor.matmul(avd[hb][i], stat,
                                     pts[hb][:, sl], start=first, stop=last)

        for c in range(CT):
            nc.scalar.dma_start(out=wo_sb[c], in_=wo[c * P:(c + 1) * P, :])
        nc.gpsimd.dma_start(out=gam_sb,
                            in_=bass.AP(tensor=gam, offset=0, ap=[[0, P], [1, D_MODEL]]))
        nc.gpsimd.dma_start(out=bet_sb,
                            in_=bass.AP(tensor=bet, offset=0, ap=[[0, P], [1, D_MODEL]]))

        # ---- attention ------------------------------------------------------
        attn_ctx = ExitStack()
        scp = attn_ctx.enter_context(tc.tile_pool(name="scp", bufs=2, space="PSUM"))
        avp = attn_ctx.enter_context(tc.tile_pool(name="avp", bufs=1, space="PSUM"))
        ptp = attn_ctx.enter_context(tc.tile_pool(name="ptp", bufs=7))
        nrm = attn_ctx.enter_context(tc.tile_pool(name="nrm", bufs=2))

        for hp in range(HP):
            avd = [[avp.tile([P, NSL], f32, tag=f"avd{hb}{i}",
                             name=f"avd{hp}_{hb}_{i}") for i in range(IS)]
                   for hb in range(2)]
            # interleaved projection work for the NEXT head pair, borrowing
            # scores-pool psum slots. One 8-matmul slab (~1.8us) per burst so
            # the ScalarE exp stream never runs dry; weight-column stripes
            # are prefetched ~10 iterations ahead of first use.
            if hp + 1 < HP:
                proj_work = {1: ("kl", 0), 3: ("k", 0), 5: ("k", 1),
                             7: ("k", 2), 9: ("k", 3), 10: ("ql", 0),
                             11: ("q", 0), 13: ("q", 1)}
            else:
                proj_work = {}
            prev_pt = None
            wc_k = None
            wc_q = None

            for j in range(JT):
                # interleaved projection work first: the borrow tile then
                # takes the psum slot freed by exp(j-1) instead of blocking
                # on this iteration's own exps, and the 8-matmul chain
                # overlaps the queued exp(j-1) backlog
                if j in proj_work:
                    kind, sl = proj_work[j]
                    if kind == "kl":
                        wc_k = load_wcol(wk, hp + 1, "wkc", attn_engs)
                    elif kind == "ql":
                        wc_q = load_wcol(wq, hp + 1, "wqc", attn_engs)
                    else:
                        borrow = scp.tile([P, QLEN], f32, tag="sc",
                                          name=f"bw{hp}_{j}")
                        wc, dst = (wc_k, kt_sb) if kind == "k" else (wc_q, qt_sb)
                        kq_group(borrow[:, 0:NSL], wc, ht_sb, sl)
                        nc.vector.tensor_copy(
                            dst[hp + 1][:, sl * NSL:(sl + 1) * NSL],
                            borrow[:, 0:NSL])
                scs = [scp.tile([P, QLEN], f32, tag="sc", name=f"sc{hp}_{j}_{hb}")
                       for hb in range(2)]
                for hb in range(2):
                    base = hb * 64
                    for i in range(IS):
                        nc.tensor.matmul(
                            scs[hb][:, i * NSL:(i + 1) * NSL],
                            kt_sb[hp][base:base + 64, j * P:(j + 1) * P],
                            qt_sb[hp][base:base + 64, i * NSL:(i + 1) * NSL],
                            start=True, stop=True, tile_position=(base, 0))
                cur_pt = []
                for hb in range(2):
                    pt_t = ptp.tile([P, QLEN], bf16, tag="pt",
                                    name=f"pt{hp}_{j}_{hb}")
                    nc.scalar.activation(pt_t, scs[hb], AF.Exp,
                                         bias=mask_sb[:, j:j + 1], scale=SCALE)
                    cur_pt.append(pt_t)

                if prev_pt is not None:
                    emit_pv(nc, v_sb, avd, hp, j - 1, prev_pt)
                prev_pt = cur_pt


            # last PV round
            emit_pv(nc, v_sb, avd, hp, JT - 1, prev_pt)

            # normalize: even head has av@0-63/den@64-127, odd head the
            # mirror. Reciprocal_approx_fast only works at base partition 0,
            # so route the den rows there (DVE cross-partition copies are
            # fine), then stt straight from psum into the bf16 avt strip.
            for hb in range(2):
                for i in range(IS):
                    sl = slice(i * NSL, (i + 1) * NSL)
                    dcp = nrm.tile([64, NSL], f32, tag="dcp",
                                   name=f"dcp{hp}_{hb}_{i}")
                    den_rows = avd[hb][i][64:P, :] if hb == 0 \
                        else avd[hb][i][0:64, :]
                    nc.vector.tensor_copy(dcp, den_rows)
                    rep = nrm.tile([64, NSL], f32, tag="rep",
                                   name=f"rep{hp}_{hb}_{i}")
                    nc.vector.reciprocal_approx_fast(out=rep, in_=dcp)
                    if hb == 0:
                        nc.vector.scalar_tensor_tensor(
                            out=avt_sb[hp][0:64, sl],
                            in0=avd[0][i][0:64, :], scalar=1.0,
                            in1=rep, op0=ALU.mult, op1=ALU.mult)
                    else:
                        rep64 = nrm.tile([P, NSL], f32, tag="rep64",
                                         name=f"rep64_{hp}_{i}")
                        nc.vector.tensor_copy(rep64[64:P, :], rep)
                        nc.vector.scalar_tensor_tensor(
                            out=avt_sb[hp][64:P, sl],
                            in0=avd[1][i][64:P, :], scalar=1.0,
                            in1=rep64[64:P, :], op0=ALU.mult, op1=ALU.mult)

        # ---- output projection + residual + layernorm -----------------------
        attn_ctx.close()
        ph1_ctx.close()

        pso = ctx.enter_context(tc.tile_pool(name="pso", bufs=8, space="PSUM"))
        lnp = ctx.enter_context(tc.tile_pool(name="lnp", bufs=3))
        lns = ctx.enter_context(tc.tile_pool(name="lns", bufs=8))
        hqp = ctx.enter_context(tc.tile_pool(name="hqp", bufs=1))

        # batch all residual loads upfront across queues so they overlap the
        # first output-projection matmuls instead of gating each tile
        hq_ts = [hqp.tile([P, D_MODEL], f32, name=f"hq{t}") for t in range(TQ)]
        for t in range(TQ):
            eng = (nc.sync, nc.scalar, nc.gpsimd)[t % 3]
            eng.dma_start(out=hq_ts[t], in_=hq[t * P:(t + 1) * P, :])

        for t in range(TQ):
            hq_t = hq_ts[t]
            xs = lnp.tile([P, D_MODEL], f32, tag="xs", name=f"xs{t}")
            sums = lns.tile([P, 2], f32, tag="sm", name=f"sm{t}")
            for m in range(2):
                ps = pso.tile([P, NSL], f32, tag="po", name=f"po{t}_{m}")
                for e in range(ET):
                    nc.tensor.matmul(ps, avt_sb[e][:, t * P:(t + 1) * P],
                                     wo_sb[e][:, m * NSL:(m + 1) * NSL],
                                     start=(e == 0), stop=(e == ET - 1))
                nc.vector.scalar_tensor_tensor(
                    out=xs[:, m * NSL:(m + 1) * NSL], in0=ps, scalar=1.0,
                    in1=hq_t[:, m * NSL:(m + 1) * NSL],
                    op0=ALU.mult, op1=ALU.add,
                    accum_out=sums[:, m:m + 1])
            # mean/var via accum sums + ACT Square pass; mean = (s0+s1)/D,
            # var = sq/D - mean^2
            sq = lns.tile([P, 2], f32, tag="sq", name=f"sq{t}")
            xsq = lnp.tile([P, D_MODEL], f32, tag="xq", name=f"xq{t}")
            for m in range(2):
                nc.scalar.activation(xsq[:, m * NSL:(m + 1) * NSL],
                                     xs[:, m * NSL:(m + 1) * NSL], AF.Square,
                                     accum_out=sq[:, m:m + 1])
            mean = lns.tile([P, 1], f32, tag="mn", name=f"mn{t}")
            nc.vector.tensor_add(mean, sums[:, 0:1], sums[:, 1:2])
            nc.vector.tensor_scalar_mul(mean, mean, 1.0 / D_MODEL)
            msq = lns.tile([P, 1], f32, tag="mq", name=f"mq{t}")
            nc.vector.tensor_mul(msq, mean, mean)
            var = lns.tile([P, 1], f32, tag="vr", name=f"vr{t}")
            nc.vector.tensor_add(var, sq[:, 0:1], sq[:, 1:2])
            nc.vector.scalar_tensor_tensor(
                out=var, in0=var, scalar=1.0 / D_MODEL, in1=msq,
                op0=ALU.mult, op1=ALU.subtract)
            std = lns.tile([P, 1], f32, tag="sd", name=f"sd{t}")
            nc.scalar.activation(std, var, AF.Sqrt, bias=eps_sb[:, 0:1])
            rstd = lns.tile([P, 1], f32, tag="rs", name=f"rs{t}")
            nc.vector.reciprocal_approx_fast(out=rstd, in_=std)
            nmr = lns.tile([P, 1], f32, tag="nm", name=f"nm{t}")
            nc.vector.tensor_scalar_mul(nmr, mean, -1.0)
            gs = lnp.tile([P, D_MODEL], f32, tag="gs", name=f"gs{t}")
            nc.vector.tensor_scalar(out=gs, in0=gam_sb,
                                    scalar1=rstd[:, 0:1], scalar2=None,
                                    op0=ALU.mult)
            xg = lnp.tile([P, D_MODEL], f32, tag="xg", name=f"xg{t}")
            nc.vector.scalar_tensor_tensor(
                out=xg, in0=xs, scalar=nmr[:, 0:1], in1=gs,
                op0=ALU.add, op1=ALU.mult)
            xn = lnp.tile([P, D_MODEL], f32, tag="xn", name=f"xn{t}")
            nc.vector.tensor_add(xn, xg, bet_sb)
            nc.sync.dma_start(out=out[t * P:(t + 1) * P, :], in_=xn)

    nc.compile()
    return nc


def _get_nc():
    if "nc" not in _CACHE:
        _CACHE["nc"] = _build()
    return _CACHE["nc"]


def _make_in_maps(inputs):
    bf = ml_dtypes.bfloat16
    h = np.asarray(inputs["h"], dtype=np.float32)
    mask = np.asarray(inputs["attn_mask"])
    Wq = np.asarray(inputs["Wq"], dtype=np.float32)
    Wkv = np.asarray(inputs["Wkv"], dtype=np.float32)
    Wo = np.asarray(inputs["Wo"], dtype=np.float32)
    gamma = np.asarray(inputs["gamma"], dtype=np.float32)
    beta = np.asarray(inputs["beta"], dtype=np.float32)

    wq_b = np.ascontiguousarray(Wq.astype(bf))
    wk_b = np.ascontiguousarray(Wkv[:, :D_MODEL].astype(bf))
    wv_b = np.ascontiguousarray(Wkv[:, D_MODEL:].astype(bf))
    wo_b = np.ascontiguousarray(Wo.astype(bf))

    in_maps = []
    for c in range(8):
        b, half = divmod(c, 2)
        hb = h[:, b, :]
        hT_b = hb.T.astype(bf)
        own = slice(half * QLEN, (half + 1) * QLEN)
        other = slice((1 - half) * QLEN, (2 - half) * QLEN)
        # own query-half first: keys are in core-local order, so the Q
        # projection can read the first half of hT uniformly on every core.
        # The mask is reordered identically; attention is key-order-invariant.
        hT_r = np.ascontiguousarray(np.concatenate(
            [hT_b[:, own], hT_b[:, other]], axis=1))
        mb_full = np.where(mask[:, b], np.float32(-1e9), np.float32(0.0))
        in_maps.append({
            "hT": hT_r,
            "hq": np.ascontiguousarray(hb[own, :]),
            "wq": wq_b, "wk": wk_b, "wv": wv_b, "wo": wo_b,
            "mb": np.ascontiguousarray(
                np.concatenate([mb_full[own], mb_full[other]])
                .reshape(JT, P).T),
            "gam": gamma, "bet": beta,
        })
    return in_maps


def _run(in_maps, **kwargs):
    from concourse.bass_utils import run_bass_kernel_spmd
    return run_bass_kernel_spmd(_get_nc(), in_maps, core_ids=list(range(8)),
                                **kwargs)


def kernel(**inputs) -> np.ndarray:
    res = _run(_make_in_maps(inputs))
    out = np.empty((SEQ, BSZ, D_MODEL), dtype=np.float32)
    for c in range(8):
        b, half = divmod(c, 2)
        out[half * QLEN:(half + 1) * QLEN, :, :][:, b, :] = res.results[c]["out"]
    return out



# revision 52
# speedup vs baseline: 1.0468x; 1.0150x over previous
"""MultiHeadAttn (post-LN, key-padding mask) Trainium2 Bass kernel, 8 cores.

Problem: h [S=2048, B=4, D=1024] f32; 16 heads x 64; key-padding mask [S, B];
out = LayerNorm(h + MHA(h)).

Sharding: core c handles batch b = c//2 and query half qh = c%2 (1024 query
rows), with all 16 heads and the full 2048-key context for that batch.
KV projections are recomputed by both cores of a batch pair (no collectives).

Per-core device pipeline (all matmuls bf16, fp32 accumulation in PSUM):
  - K^T/Q^T proj: stationary Wk/Wq column tiles, moving h^T -> [e, t] layout.
  - V proj: stationary h^T tiles, moving Wv -> natural [t, e] layout, stored
    as [V_h | ones64] per head (128-wide stationary).
  - Attention per head pair: scores^T [j,i] via row-paired matmuls (two heads
    in row strips 0-63 / 64-127 of the PE array), exp via ScalarE with the
    key-padding bias as a per-partition bias and 1/sqrt(dh) as the scale,
    PV with the [V|ones] stationary (M=128: rows 0-63 attn_vec, rows 64-127
    the softmax denominator replicated), then normalize via cross-partition
    copy + reciprocal_approx_fast + one stt per head.
  - Output proj: stationary attn_vec^T tiles, moving Wo; residual add + LN
    (stt-accum sums + ACT Square pass) with batched residual prefetch.
Next head pair's K/Q projections are interleaved into the attention loop as
single 8-matmul bursts (borrowing a scores-pool PSUM slot ahead of the
iteration's own scores so they ride the freed exp(j-1) bank), with
weight-column stripes prefetched ~10 iterations early as 8 per-c DMAs.
Startup interleaves ht[c]/wvh[c] DMA pairs across queues so the V-projection
c-chain starts consuming at ~4us and warms the PE clock early.
"""
import numpy as np
import ml_dtypes

N_HEAD, D_MODEL, D_HEAD = 16, 1024, 64
SEQ, BSZ = 2048, 4
QLEN = SEQ // 2
SCALE = 1.0 / D_HEAD ** 0.5
LN_EPS = 1e-5
P = 128
NSL = 512                   # matmul moving-operand slab (one PSUM bank fp32)
CT = D_MODEL // P           # 8 contraction tiles
ET = D_MODEL // P           # 8 e-tiles (2 heads each)
JT = SEQ // P               # 16 key tiles
JS = SEQ // NSL             # 4 key slabs
IS = QLEN // NSL            # 2 query slabs
TQ = QLEN // P              # 8 query-row tiles
HP = N_HEAD // 2            # 8 head pairs

_CACHE = {}


def _build():
    from contextlib import ExitStack
    import concourse.bass as bass
    import concourse.mybir as mybir
    import concourse.tile as tile
    from concourse import bacc

    dt = mybir.dt
    f32, bf16 = dt.float32, dt.bfloat16
    AF = mybir.ActivationFunctionType
    ALU = mybir.AluOpType

    nc = bacc.Bacc(None, target_bir_lowering=False)

    hT = nc.dram_tensor("hT", [D_MODEL, SEQ], bf16, kind="ExternalInput")
    hq = nc.dram_tensor("hq", [QLEN, D_MODEL], f32, kind="ExternalInput")
    wq = nc.dram_tensor("wq", [D_MODEL, D_MODEL], bf16, kind="ExternalInput")
    wk = nc.dram_tensor("wk", [D_MODEL, D_MODEL], bf16, kind="ExternalInput")
    wv = nc.dram_tensor("wv", [D_MODEL, D_MODEL], bf16, kind="ExternalInput")
    wo = nc.dram_tensor("wo", [D_MODEL, D_MODEL], bf16, kind="ExternalInput")
    mb = nc.dram_tensor("mb", [P, JT], f32, kind="ExternalInput")
    gam = nc.dram_tensor("gam", [D_MODEL], f32, kind="ExternalInput")
    bet = nc.dram_tensor("bet", [D_MODEL], f32, kind="ExternalInput")
    out = nc.dram_tensor("out", [QLEN, D_MODEL], f32, kind="ExternalOutput")

    with tile.TileContext(nc) as tc, ExitStack() as ctx:
        persist = ctx.enter_context(tc.tile_pool(name="persist", bufs=1))

        avt_sb = [persist.tile([P, QLEN], bf16, name=f"avt{e}") for e in range(ET)]
        mask_sb = persist.tile([P, JT], f32, name="mask")
        eps_sb = persist.tile([P, 1], f32, name="eps")

        nc.vector.memset(eps_sb, LN_EPS)

        nc.gpsimd.dma_start(out=mask_sb, in_=mb[0:P, :])

        # ---- phase-3 weights: load early into the region wvp freed ---------
        w3p = ctx.enter_context(tc.tile_pool(name="w3p", bufs=1))
        wo_sb = [w3p.tile([P, D_MODEL], bf16, name=f"wo{c}") for c in range(CT)]
        gam_sb = w3p.tile([P, D_MODEL], f32, name="gamr")
        bet_sb = w3p.tile([P, D_MODEL], f32, name="betr")

        # ---- phase 1 scope: h^T residency + streamed W columns + K/Q/V data
        # (all freed before the output-projection/LN phase)
        ph1_ctx = ExitStack()
        ph1 = ph1_ctx.enter_context(tc.tile_pool(name="ph1", bufs=1))
        ht_sb = [ph1.tile([P, SEQ], bf16, name=f"ht{c}") for c in range(CT)]
        kt_sb = [ph1.tile([P, SEQ], bf16, name=f"kt{e}") for e in range(ET)]
        qt_sb = [ph1.tile([P, QLEN], bf16, name=f"qt{e}") for e in range(ET)]
        # per head pair: [V_even | ones64 | V_odd]; the even head's stationary
        # window is cols 0:128 ([V|ones] -> av rows 0-63, den rows 64-127),
        # the odd head's is cols 64:192 ([ones|V] -> den rows 0-63, av rows
        # 64-127). One shared ones block per pair keeps SBUF in budget.
        v_sb = [ph1.tile([P, HP, 3, D_HEAD], bf16, name=f"v{t}")
                for t in range(JT)]
        for t in range(JT):
            nc.gpsimd.memset(v_sb[t][:, :, 1, :], 1.0)

        wcol = ph1_ctx.enter_context(tc.tile_pool(name="wcol", bufs=2))
        wcolq = ph1_ctx.enter_context(tc.tile_pool(name="wcolq", bufs=1))

        # one weight column-stripe as 8 contiguous per-c DMAs spread over
        # queues (a single strided dma_start here costs ~1024 descriptors on
        # one queue, ~38us serial)
        wcol_engs = [nc.sync, nc.scalar, nc.gpsimd]

        def load_wcol(w, e, tag, engs=None):
            engs = engs or wcol_engs
            pool = wcolq if tag == "wqc" else wcol
            wc = pool.tile([P, CT, P], bf16, tag=tag, name=f"{tag}{e}")
            for c in range(CT):
                engs[c % len(engs)].dma_start(
                    out=wc[:, c, :],
                    in_=w[c * P:(c + 1) * P, e * P:(e + 1) * P])
            return wc

        # mid-attention loads must stay off the Scalar (ACT) queue: a
        # dma_start there blocks the exp stream behind pool semaphores
        attn_engs = [nc.sync, nc.gpsimd]

        # startup DMA priority: V proj runs first and its c-chain consumes
        # ht[c]+wvh0[c] in order, so interleave those pairs across queues --
        # the PE then starts at ~4us (and warms its clock) instead of
        # waiting for the full h^T load.

        def kq_group(ps_ap, wc, moving, sl):
            """8 accumulating matmuls: one K/Q-proj output group into psum."""
            for c in range(CT):
                nc.tensor.matmul(ps_ap, wc[:, c, :],
                                 moving[c][:, sl * NSL:(sl + 1) * NSL],
                                 start=(c == 0), stop=(c == CT - 1))

        # prephase: K(0), Q(0), V (own pools, closed before attention)
        with tc.tile_pool(name="wvp", bufs=1) as wvp, \
             tc.tile_pool(name="psA", bufs=6, space="PSUM") as psA:
            wvh0 = [wvp.tile([P, NSL], bf16, tag=f"wvh{c}", name=f"wv0_{c}")
                    for c in range(CT)]
            for c in range(CT):
                eng = wcol_engs[c % 3]
                eng.dma_start(out=ht_sb[c], in_=hT[c * P:(c + 1) * P, :])
                eng.dma_start(out=wvh0[c],
                              in_=wv[c * P:(c + 1) * P, 0:NSL])
            wc0 = load_wcol(wk, 0, "wkc")
            wcq0 = load_wcol(wq, 0, "wqc")
            def v_half(es, wvh):
                for t in range(JT):
                    ps = psA.tile([P, NSL], f32, tag="psa", name=f"psv{t}_{es}")
                    for c in range(CT):
                        nc.tensor.matmul(ps, ht_sb[c][:, t * P:(t + 1) * P],
                                         wvh[c],
                                         start=(c == 0), stop=(c == CT - 1))
                    psr = ps[:, :].rearrange("p (pr par d) -> p pr par d",
                                             par=2, d=D_HEAD)
                    nc.vector.tensor_copy(
                        v_sb[t][:, es * 4:(es + 1) * 4, 0, :], psr[:, :, 0, :])
                    nc.vector.tensor_copy(
                        v_sb[t][:, es * 4:(es + 1) * 4, 2, :], psr[:, :, 1, :])

            # order: V(es=0), then K0/Q0 (fills the Wv second-half reload
            # gap with useful matmuls), then V(es=1)
            v_half(0, wvh0)
            wvh1 = [wvp.tile([P, NSL], bf16, tag=f"wvh{c}",
                             name=f"wv1_{c}") for c in range(CT)]
            for c in range(CT):
                wcol_engs[c % 3].dma_start(
                    out=wvh1[c], in_=wv[c * P:(c + 1) * P, NSL:2 * NSL])
            wc = wc0
            for j in range(JS):
                ps = psA.tile([P, NSL], f32, tag="psa", name=f"psk0_{j}")
                kq_group(ps, wc, ht_sb, j)
                nc.vector.tensor_copy(kt_sb[0][:, j * NSL:(j + 1) * NSL], ps)
            wc = wcq0
            for i in range(IS):
                ps = psA.tile([P, NSL], f32, tag="psa", name=f"psq0_{i}")
                kq_group(ps, wc, ht_sb, i)
                nc.vector.tensor_copy(qt_sb[0][:, i * NSL:(i + 1) * NSL], ps)
            v_half(1, wvh1)

        def emit_pv(nc, v_sb, avd, hp, j, pts):
            first, last = (j == 0), (j == JT - 1)
            for hb in range(2):
                stat = v_sb[j][:, hp, hb:hb + 2, :]
                for i in range(IS):
                    sl = slice(i * NSL, (i + 1) * NSL)
                    nc.tensor.matmul(avd[hb][i], stat,
                                     pts[hb][:, sl], start=first, stop=last)

        for c in range(CT):
            nc.scalar.dma_start(out=wo_sb[c], in_=wo[c * P:(c + 1) * P, :])
        nc.gpsimd.dma_start(out=gam_sb,
                            in_=bass.AP(tensor=gam, offset=0, ap=[[0, P], [1, D_MODEL]]))
        nc.gpsimd.dma_start(out=bet_sb,
                            in_=bass.AP(tensor=bet, offset=0, ap=[[0, P], [1, D_MODEL]]))

        # ---- attention ------------------------------------------------------
        attn_ctx = ExitStack()
        scp = attn_ctx.enter_context(tc.tile_pool(name="scp", bufs=2, space="PSUM"))
        avp = attn_ctx.enter_context(tc.tile_pool(name="avp", bufs=1, space="PSUM"))
        ptp = attn_ctx.enter_context(tc.tile_pool(name="ptp", bufs=7))
        nrm = attn_ctx.enter_context(tc.tile_pool(name="nrm", bufs=2))

        for hp in range(HP):
            avd = [[avp.tile([P, NSL], f32, tag=f"avd{hb}{i}",
                             name=f"avd{hp}_{hb}_{i}") for i in range(IS)]
                   for hb in range(2)]
            # interleaved projection work for the NEXT head pair, borrowing
            # scores-pool psum slots. One 8-matmul slab (~1.8us) per burst so
            # the ScalarE exp stream never runs dry; weight-column stripes
            # are prefetched ~10 iterations ahead of first use.
            if hp + 1 < HP:
                proj_work = {1: ("kl", 0), 3: ("k", 0), 5: ("k", 1),
                             7: ("k", 2), 9: ("k", 3), 10: ("ql", 0),
                             11: ("q", 0), 13: ("q", 1)}
            else:
                proj_work = {}
            prev_pt = None
            wc_k = None
            wc_q = None

            for j in range(JT):
                scs = [scp.tile([P, QLEN], f32, tag="sc", name=f"sc{hp}_{j}_{hb}")
                       for hb in range(2)]
                for hb in range(2):
                    base = hb * 64
                    for i in range(IS):
                        nc.tensor.matmul(
                            scs[hb][:, i * NSL:(i + 1) * NSL],
                            kt_sb[hp][base:base + 64, j * P:(j + 1) * P],
                            qt_sb[hp][base:base + 64, i * NSL:(i + 1) * NSL],
                            start=True, stop=True, tile_position=(base, 0))
                # interleaved projection work after the scores: the borrow
                # then waits on this iteration's exp(hb0) (covered by the
                # ACT backlog) instead of delaying the hb1 score tile
                if j in proj_work:
                    kind, sl = proj_work[j]
                    if kind == "kl":
                        wc_k = load_wcol(wk, hp + 1, "wkc", attn_engs)
                    elif kind == "ql":
                        wc_q = load_wcol(wq, hp + 1, "wqc", attn_engs)
                    else:
                        borrow = scp.tile([P, QLEN], f32, tag="sc",
                                          name=f"bw{hp}_{j}")
                        wc, dst = (wc_k, kt_sb) if kind == "k" else (wc_q, qt_sb)
                        kq_group(borrow[:, 0:NSL], wc, ht_sb, sl)
                        nc.vector.tensor_copy(
                            dst[hp + 1][:, sl * NSL:(sl + 1) * NSL],
                            borrow[:, 0:NSL])
                cur_pt = []
                for hb in range(2):
                    pt_t = ptp.tile([P, QLEN], bf16, tag="pt",
                                    name=f"pt{hp}_{j}_{hb}")
                    nc.scalar.activation(pt_t, scs[hb], AF.Exp,
                                         bias=mask_sb[:, j:j + 1], scale=SCALE)
                    cur_pt.append(pt_t)

                if prev_pt is not None:
                    emit_pv(nc, v_sb, avd, hp, j - 1, prev_pt)
                prev_pt = cur_pt


            # last PV round
            emit_pv(nc, v_sb, avd, hp, JT - 1, prev_pt)

            # normalize: even head has av@0-63/den@64-127, odd head the
            # mirror. Reciprocal_approx_fast only works at base partition 0,
            # so route the den rows there (DVE cross-partition copies are
            # fine), then stt straight from psum into the bf16 avt strip.
            for hb in range(2):
                for i in range(IS):
                    sl = slice(i * NSL, (i + 1) * NSL)
                    dcp = nrm.tile([64, NSL], f32, tag="dcp",
                                   name=f"dcp{hp}_{hb}_{i}")
                    den_rows = avd[hb][i][64:P, :] if hb == 0 \
                        else avd[hb][i][0:64, :]
                    nc.vector.tensor_copy(dcp, den_rows)
                    rep = nrm.tile([64, NSL], f32, tag="rep",
                                   name=f"rep{hp}_{hb}_{i}")
                    nc.vector.reciprocal_approx_fast(out=rep, in_=dcp)
                    if hb == 0:
                        nc.vector.scalar_tensor_tensor(
                            out=avt_sb[hp][0:64, sl],
                            in0=avd[0][i][0:64, :], scalar=1.0,
                            in1=rep, op0=ALU.mult, op1=ALU.mult)
                    else:
                        rep64 = nrm.tile([P, NSL], f32, tag="rep64",
                                         name=f"rep64_{hp}_{i}")
                        nc.vector.tensor_copy(rep64[64:P, :], rep)
                        nc.vector.scalar_tensor_tensor(
                            out=avt_sb[hp][64:P, sl],
                            in0=avd[1][i][64:P, :], scalar=1.0,
                            in1=rep64[64:P, :], op0=ALU.mult, op1=ALU.mult)

        # ---- output projection + residual + layernorm -----------------------
        attn_ctx.close()
        ph1_ctx.close()

        pso = ctx.enter_context(tc.tile_pool(name="pso", bufs=8, space="PSUM"))
        lnp = ctx.enter_context(tc.tile_pool(name="lnp", bufs=3))
        lns = ctx.enter_context(tc.tile_pool(name="lns", bufs=8))
        hqp = ctx.enter_context(tc.tile_pool(name="hqp", bufs=1))

        # batch all residual loads upfront across queues so they overlap the
        # first output-projection matmuls instead of gating each tile
        hq_ts = [hqp.tile([P, D_MODEL], f32, name=f"hq{t}") for t in range(TQ)]
        for t in range(TQ):
            eng = (nc.sync, nc.scalar, nc.gpsimd)[t % 3]
            eng.dma_start(out=hq_ts[t], in_=hq[t * P:(t + 1) * P, :])

        for t in range(TQ):
            hq_t = hq_ts[t]
            xs = lnp.tile([P, D_MODEL], f32, tag="xs", name=f"xs{t}")
            sums = lns.tile([P, 2], f32, tag="sm", name=f"sm{t}")
            for m in range(2):
                ps = pso.tile([P, NSL], f32, tag="po", name=f"po{t}_{m}")
                for e in range(ET):
                    nc.tensor.matmul(ps, avt_sb[e][:, t * P:(t + 1) * P],
                                     wo_sb[e][:, m * NSL:(m + 1) * NSL],
                                     start=(e == 0), stop=(e == ET - 1))
                nc.vector.scalar_tensor_tensor(
                    out=xs[:, m * NSL:(m + 1) * NSL], in0=ps, scalar=1.0,
                    in1=hq_t[:, m * NSL:(m + 1) * NSL],
                    op0=ALU.mult, op1=ALU.add,
                    accum_out=sums[:, m:m + 1])
            # mean/var via accum sums + ACT Square pass; mean = (s0+s1)/D,
            # var = sq/D - mean^2
            sq = lns.tile([P, 2], f32, tag="sq", name=f"sq{t}")
            xsq = lnp.tile([P, D_MODEL], f32, tag="xq", name=f"xq{t}")
            for m in range(2):
                nc.scalar.activation(xsq[:, m * NSL:(m + 1) * NSL],
                                     xs[:, m * NSL:(m + 1) * NSL], AF.Square,
                                     accum_out=sq[:, m:m + 1])
            mean = lns.tile([P, 1], f32, tag="mn", name=f"mn{t}")
            nc.vector.tensor_add(mean, sums[:, 0:1], sums[:, 1:2])
            nc.vector.tensor_scalar_mul(mean, mean, 1.0 / D_MODEL)
            msq = lns.tile([P, 1], f32, tag="mq", name=f"mq{t}")
            nc.vector.tensor_mul(msq, mean, mean)
            var = lns.tile([P, 1], f32, tag="vr", name=f"vr{t}")
            nc.vector.tensor_add(var, sq[:, 0:1], sq[:, 1:2])
            nc.vector.scalar_tensor_tensor(
                out=var, in0=var, scalar=1.0 / D_MODEL, in1=msq,
                op0=ALU.mult, op1=ALU.subtract)
            std = lns.tile([P, 1], f32, tag="sd", name=f"sd{t}")
            nc.scalar.activation(std, var, AF.Sqrt, bias=eps_sb[:, 0:1])
            rstd = lns.tile([P, 1], f32, tag="rs", name=f"rs{t}")
            nc.vector.reciprocal_approx_fast(out=rstd, in_=std)
            nmr = lns.tile([P, 1], f32, tag="nm", name=f"nm{t}")
            nc.vector.tensor_scalar_mul(nmr, mean, -1.0)
            gs = lnp.tile([P, D_MODEL], f32, tag="gs", name=f"gs{t}")
            nc.vector.tensor_scalar(out=gs, in0=gam_sb,
                                    scalar1=rstd[:, 0:1], scalar2=None,
                                    op0=ALU.mult)
            xg = lnp.tile([P, D_MODEL], f32, tag="xg", name=f"xg{t}")
            nc.vector.scalar_tensor_tensor(
                out=xg, in0=xs, scalar=nmr[:, 0:1], in1=gs,
                op0=ALU.add, op1=ALU.mult)
            xn = lnp.tile([P, D_MODEL], f32, tag="xn", name=f"xn{t}")
            nc.vector.tensor_add(xn, xg, bet_sb)
            nc.sync.dma_start(out=out[t * P:(t + 1) * P, :], in_=xn)

    nc.compile()
    return nc


def _get_nc():
    if "nc" not in _CACHE:
        _CACHE["nc"] = _build()
    return _CACHE["nc"]


def _make_in_maps(inputs):
    bf = ml_dtypes.bfloat16
    h = np.asarray(inputs["h"], dtype=np.float32)
    mask = np.asarray(inputs["attn_mask"])
    Wq = np.asarray(inputs["Wq"], dtype=np.float32)
    Wkv = np.asarray(inputs["Wkv"], dtype=np.float32)
    Wo = np.asarray(inputs["Wo"], dtype=np.float32)
    gamma = np.asarray(inputs["gamma"], dtype=np.float32)
    beta = np.asarray(inputs["beta"], dtype=np.float32)

    wq_b = np.ascontiguousarray(Wq.astype(bf))
    wk_b = np.ascontiguousarray(Wkv[:, :D_MODEL].astype(bf))
    wv_b = np.ascontiguousarray(Wkv[:, D_MODEL:].astype(bf))
    wo_b = np.ascontiguousarray(Wo.astype(bf))

    in_maps = []
    for c in range(8):
        b, half = divmod(c, 2)
        hb = h[:, b, :]
        hT_b = hb.T.astype(bf)
        own = slice(half * QLEN, (half + 1) * QLEN)
        other = slice((1 - half) * QLEN, (2 - half) * QLEN)
        # own query-half first: keys are in core-local order, so the Q
        # projection can read the first half of hT uniformly on every core.
        # The mask is reordered identically; attention is key-order-invariant.
        hT_r = np.ascontiguousarray(np.concatenate(
            [hT_b[:, own], hT_b[:, other]], axis=1))
        mb_full = np.where(mask[:, b], np.float32(-1e9), np.float32(0.0))
        in_maps.append({
            "hT": hT_r,
            "hq": np.ascontiguousarray(hb[own, :]),
            "wq": wq_b, "wk": wk_b, "wv": wv_b, "wo": wo_b,
            "mb": np.ascontiguousarray(
                np.concatenate([mb_full[own], mb_full[other]])
                .reshape(JT, P).T),
            "gam": gamma, "bet": beta,
        })
    return in_maps


def _run(in_maps, **kwargs):
    from concourse.bass_utils import run_bass_kernel_spmd
    return run_bass_kernel_spmd(_get_nc(), in_maps, core_ids=list(range(8)),
                                **kwargs)


def kernel(**inputs) -> np.ndarray:
    res = _run(_make_in_maps(inputs))
    out = np.empty((SEQ, BSZ, D_MODEL), dtype=np.float32)
    for c in range(8):
        b, half = divmod(c, 2)
        out[half * QLEN:(half + 1) * QLEN, :, :][:, b, :] = res.results[c]["out"]
    return out

